# revision 1
# baseline (speedup 1.0000x reference)
"""CayleyNet GNN kernel for Trainium2 — 8 NeuronCores via bass SPMD.

Sharding (per the graph-parallel hint): nodes are band-sorted by in-degree
and dealt round-robin to 8 cores, so every core owns an equal slice of
destination nodes with matched degree profiles (one shared SPMD program).
Each of the 30 sparse transfers (out[dst] += z[src] over 800k edges) runs
on device as:
  - CSR-by-destination gather: per-dst slot grids (tiles of 128 dsts,
    depth-padded); gpsimd dma_gather fetches 256-byte z[src] rows from a
    replicated DRAM node array (the full-exchange "halo"); pad slots point
    at an all-zero row. int16 index reach is handled with two base views
    (lo: rows < 32001, hi: view based at row 18000).
  - DVE segment tensor_reduce sums each dst's slots -> [dst, 128ch] f32.
The cheap per-node complex scalings between transfers (CayleyNet's edge
weights depend on a single endpoint, so each SpMM factorizes into an
unweighted transfer plus per-node complex scales) and the tiny pooling
head ([50000,64] -> [10,10]) run on host between device calls.
"""
import numpy as np
import ml_dtypes

N = 50000
E = 800000
H = 64
G_GRAPHS = 10
NPG = N // G_GRAPHS
R = 3
KK = 4
NCONV = 2
OUT = 10
RATIO = 0.9

NCORES = 8
NTILE = 128
SL = 6272                # nodes per core slice (49 tiles)
TPC = SL // NTILE        # 49
NPAD = SL * NCORES       # 50176
ZROWS = NPAD + 2         # zero row at 0 and NPAD+1
SPLIT = 32000            # relabeled rows < SPLIT -> lo view
HIBASE = 18000
HIPAD = ZROWS - 1 - HIBASE
CHUNK = 1024
SEGROWS = 96
GMAX = 12
DQ = 4

bf16 = ml_dtypes.bfloat16
_CACHE = {}


# --------------------------------------------------------------------------
# host graph preprocessing
# --------------------------------------------------------------------------

def _relabel(row, col):
    """Band-sort nodes by P-direction (dst=col) degree, deal round-robin to
    cores. Returns new_of_old [N] -> relabeled id in [0, NPAD)."""
    degc = np.bincount(col, minlength=NPAD)  # includes pad ids unused
    degc = degc[:NPAD]
    order = np.argsort(-degc[:NPAD], kind="stable")  # nodes by desc degree
    # order includes pad ids (zero degree, at the end) — order has NPAD ids:
    # real nodes 0..N-1 plus pads N..NPAD-1
    new_of_old = np.empty(NPAD, np.int64)
    # band rank b -> core b%8, local b//8 -> relabeled id core*SL + local
    b = np.arange(NPAD)
    new_id = (b % NCORES) * SL + b // NCORES
    new_of_old[order] = new_id
    return new_of_old


def _common_plan(src, dst):
    """Common (across cores) CSR plan for one direction. src/dst are
    relabeled endpoint arrays over all E edges. Returns plan plus per-core
    int16 idx arrays."""
    dst_core = dst // SL
    dst_loc = dst % SL
    is_lo = src < SPLIT

    # per (core, local dst) degrees
    deg = np.zeros((NCORES, SL), np.int64)
    dlo = np.zeros((NCORES, SL), np.int64)
    np.add.at(deg, (dst_core, dst_loc), 1)
    np.add.at(dlo, (dst_core, dst_loc), is_lo.astype(np.int64))
    dhi = deg - dlo

    # common per-tile lo/hi depths = max over cores and tile members
    DLo = np.maximum(1, dlo.reshape(NCORES, TPC, NTILE).max(axis=(0, 2)))
    DHi = np.maximum(1, dhi.reshape(NCORES, TPC, NTILE).max(axis=(0, 2)))
    Dtot = ((DLo + DHi + DQ - 1) // DQ) * DQ
    assert Dtot.max() <= SEGROWS, f"tile depth {Dtot.max()}"

    # segments: consecutive tiles, G*D <= SEGROWS, G <= GMAX
    segs = []
    total = 0
    t = 0
    tile_pos0 = np.zeros(TPC, np.int64)
    while t < TPC:
        g, D = 1, int(Dtot[t])
        while (t + g < TPC and g < GMAX
               and max(D, int(Dtot[t + g])) * (g + 1) <= SEGROWS):
            D = max(D, int(Dtot[t + g]))
            g += 1
        segs.append((total, t, g, D))
        for k in range(g):
            tile_pos0[t + k] = total + k * D * NTILE
        total += g * D * NTILE
        t += g

    # call cuts: per tile, lo rows [0, DLo_t), hi rows [DLo_t, D_seg)
    cuts = []
    for (p0, t0, g, D) in segs:
        for k in range(g):
            tp = p0 + k * D * NTILE
            dl = int(DLo[t0 + k])
            cuts.append((tp, tp + dl * NTILE, False))
            cuts.append((tp + dl * NTILE, tp + D * NTILE, True))
    calls = []
    for (a, b, hi) in cuts:
        p = a
        while p < b:
            n = min(CHUNK, b - p)
            calls.append([p, n, hi])
            p += n
    merged = []
    for c in calls:
        if (merged and merged[-1][2] == c[2]
                and merged[-1][0] + merged[-1][1] == c[0]
                and merged[-1][1] + c[1] <= CHUNK):
            merged[-1][1] += c[1]
        else:
            merged.append(list(c))

    # per-core idx arrays
    es = np.lexsort((np.where(is_lo, 0, 1), dst))
    s_src, s_dst, s_lo = src[es], dst[es], is_lo[es]
    s_core, s_loc = s_dst // SL, s_dst % SL
    cdeg_flat = deg.reshape(-1)
    starts = np.zeros(NCORES * SL + 1, np.int64)
    np.cumsum(cdeg_flat, out=starts[1:])
    flat = s_core * SL + s_loc
    slot = np.arange(len(s_dst)) - starts[flat]
    t_of = s_loc // NTILE
    d_local = s_loc % NTILE
    dlo_e = dlo.reshape(-1)[flat]
    sit = np.where(s_lo, slot, DLo[t_of] + (slot - dlo_e))
    pos = tile_pos0[t_of] + sit * NTILE + d_local
    val = np.where(s_lo, s_src + 1, s_src + 1 - HIBASE)

    hi_mask = np.zeros(total, bool)
    for (a, b, hi) in cuts:
        if hi:
            hi_mask[a:b] = True
    base_idx = np.where(hi_mask, HIPAD, 0).astype(np.int64)

    idxs = []
    for c in range(NCORES):
        arr = base_idx.copy()
        mc = s_core == c
        arr[pos[mc]] = val[mc]
        assert arr.min() >= 0 and arr.max() < 32768
        idxs.append(np.tile(arr.reshape(-1, 16).T.astype(np.int16), (8, 1)))

    return {"segs": segs, "calls": [tuple(c) for c in merged], "total": total,
            "idx": np.stack(idxs)}


# --------------------------------------------------------------------------
# device program: one sparse transfer (gather + segment reduce)
# --------------------------------------------------------------------------

def _build_transfer_nc(plan):
    import concourse.bacc as bacc
    import concourse.mybir as mybir
    dt = mybir.dt
    nc = bacc.Bacc("TRN2", debug=False)

    total = plan["total"]
    Zin = nc.dram_tensor("Zin", [ZROWS, 128], dt.bfloat16, kind="ExternalInput")
    IDX = nc.dram_tensor("IDX", [128, total // 16], dt.int16, kind="ExternalInput")
    TOUT = nc.dram_tensor("TOUT", [SL, 128], dt.bfloat16, kind="ExternalOutput")

    with (
        nc.Block() as block,
        nc.sbuf_tensor("stg", [128, 2, SEGROWS, 128], dt.bfloat16) as stg,
        nc.sbuf_tensor("red", [128, 2, GMAX * 128], dt.float32) as red,
        nc.sbuf_tensor("redh", [128, 2, GMAX * 128], dt.bfloat16) as redh,
        nc.sbuf_tensor("ix", [128, total // 16], dt.int16) as ix,
        nc.semaphore("s_in") as s_in,
        nc.semaphore("s_g") as s_g,
        nc.semaphore("s_r") as s_r,
        nc.semaphore("s_st") as s_st,
    ):
        segs, calls = plan["segs"], plan["calls"]
        # precompute per-seg call ranges and counters
        seg_calls = []
        ci = 0
        for (p0, t0, g, D) in segs:
            npos = g * D * NTILE
            mine = []
            while ci < len(calls) and calls[ci][0] < p0 + npos:
                mine.append(calls[ci])
                ci += 1
            seg_calls.append(mine)
        gcum = np.cumsum([len(m) for m in seg_calls])

        @block.gpsimd
        def _(gp):
            gp.dma_start(ix[:], IDX[:]).then_inc(s_in, 16)
            gp.wait_ge(s_in, 16)
            for si, ((p0, t0, g, D), mine) in enumerate(zip(segs, seg_calls)):
                sb = si % 2
                if si >= 2:
                    gp.wait_ge(s_r, si - 1)   # reduce of seg si-2 done
                for (ca, cn, chi) in mine:
                    iv = ix[:, ca // 16:(ca + cn) // 16]
                    srow = (ca - p0) // 128
                    sv = stg[:, sb, srow:srow + cn // 128, :]
                    base = HIBASE if chi else 0
                    gp.dma_gather(
                        sv, Zin[base:ZROWS, :], iv, cn, cn, 128,
                    ).then_inc(s_g, 16)

        @block.vector
        def _(ve):
            for si, (p0, t0, g, D) in enumerate(segs):
                sb = si % 2
                ve.wait_ge(s_g, 16 * int(gcum[si]))
                if si >= 2:
                    ve.wait_ge(s_st, 16 * (si - 1))  # red buf reuse
                inap = stg[:, sb, 0:g * D, :].rearrange(
                    "p (g r) c -> p g r c", g=g).transpose([0, 1, 3, 2])
                outap = red[:, sb, 0:g * 128].rearrange(
                    "p (g c) -> p g c", g=g)
                ve.tensor_reduce(
                    outap, inap, mybir.AxisListType.X, mybir.AluOpType.add,
                )
                ve.tensor_copy(
                    redh[:, sb, 0:g * 128], red[:, sb, 0:g * 128],
                ).then_inc(s_r, 1)

        @block.sync
        def _(sp):
            for si, (p0, t0, g, D) in enumerate(segs):
                sb = si % 2
                sp.wait_ge(s_r, si + 1)
                r0 = t0 * NTILE
                sp.dma_start(
                    TOUT[r0:r0 + g * NTILE, :].rearrange(
                        "(a p) c -> p a c", p=128),
                    redh[:, sb, 0:g * 128].rearrange("p (a c) -> p a c", c=128),
                ).then_inc(s_st, 16)
            sp.wait_ge(s_st, 16 * len(segs))

    nc.compile()
    return nc


def _make_runner(nc, n_cores=NCORES, replicated_names=()):
    """Reusable jitted SPMD runner (mirrors bass2jax.run_bass_via_pjrt)."""
    import jax
    from jax.sharding import Mesh, PartitionSpec, NamedSharding
    from jax.experimental.shard_map import shard_map
    from concourse import mybir
    from concourse.bass2jax import (
        _bass_exec_p, install_neuronx_cc_hook, partition_id_tensor)

    install_neuronx_cc_hook()
    pname = nc.partition_id_tensor.name if nc.partition_id_tensor else None
    in_names, out_names, out_avals, zero_outs = [], [], [], []
    for alloc in nc.m.functions[0].allocations:
        if not isinstance(alloc, mybir.MemoryLocationSet):
            continue
        name = alloc.memorylocations[0].name
        if alloc.kind == "ExternalInput":
            if name != pname:
                in_names.append(name)
        elif alloc.kind == "ExternalOutput":
            shape = tuple(alloc.tensor_shape)
            dtype = mybir.dt.np(alloc.dtype)
            out_names.append(name)
            out_avals.append(jax.core.ShapedArray(shape, dtype))
            zero_outs.append(np.zeros(shape, dtype))
    n_params, n_outs = len(in_names), len(out_avals)
    all_in = list(in_names) + list(out_names) + ([pname] if pname else [])

    def _body(*args):
        operands = list(args)
        if pname is not None:
            operands.append(partition_id_tensor())
        outs = _bass_exec_p.bind(
            *operands, out_avals=tuple(out_avals), in_names=tuple(all_in),
            out_names=tuple(out_names), lowering_input_output_aliases=(),
            sim_require_finite=True, sim_require_nnan=True, nc=nc)
        return tuple(outs)

    try:
        devices = jax.devices("axon")[:n_cores]
    except Exception:
        devices = jax.devices()[:n_cores]
    mesh = Mesh(np.asarray(devices), ("core",))
    repl = set(replicated_names)
    in_specs = tuple(
        (PartitionSpec() if n in repl else PartitionSpec("core"))
        for n in in_names
    ) + (PartitionSpec("core"),) * n_outs
    sharded = jax.jit(
        shard_map(_body, mesh=mesh,
                  in_specs=in_specs,
                  out_specs=(PartitionSpec("core"),) * n_outs,
                  check_rep=False),
        keep_unused=True)

    from jax.sharding import NamedSharding
    sh = NamedSharding(mesh, PartitionSpec("core"))
    sh_rep = NamedSharding(mesh, PartitionSpec())
    dev_cache = {}

    def run(per_core_inputs, cache_names=()):
        concat_in = []
        for name in in_names:
            if name in dev_cache:
                concat_in.append(dev_cache[name])
                continue
            if name in repl:
                a = np.ascontiguousarray(np.asarray(per_core_inputs[0][name]))
                a = jax.device_put(a, sh_rep)
            else:
                a = np.ascontiguousarray(np.concatenate(
                    [np.asarray(per_core_inputs[c][name])
                     for c in range(n_cores)], axis=0))
                a = jax.device_put(a, sh)
            if name in cache_names:
                dev_cache[name] = a
            concat_in.append(a)
        if "_zeros" not in dev_cache:
            dev_cache["_zeros"] = [
                jax.device_put(
                    np.zeros((n_cores * z.shape[0], *z.shape[1:]), z.dtype), sh)
                for z in zero_outs
            ]
        concat_zero = dev_cache["_zeros"]
        outs = sharded(*concat_in, *concat_zero)
        outs = [np.asarray(a) for a in outs]
        return [
            {name: outs[i].reshape(n_cores, *out_avals[i].shape)[c]
             for i, name in enumerate(out_names)}
            for c in range(n_cores)
        ]
    return run


# --------------------------------------------------------------------------
# host orchestration of the 30 transfers
# --------------------------------------------------------------------------

def _transfer_dev(runner, plan, z):
    """z: [NPAD, 128] f32 (r|i). Returns t[NPAD, 128] f32 = sum over edges."""
    zf = np.zeros((ZROWS, 128), bf16)
    zf[1:NPAD + 1] = z.astype(bf16)
    maps = [{"Zin": zf, "IDX": plan["idx"][c]} for c in range(NCORES)]
    import time as _time
    t0 = _time.perf_counter()
    res = runner(maps, cache_names=("IDX",))
    _CACHE.setdefault("dev_times", []).append(_time.perf_counter() - t0)
    out = np.empty((NPAD, 128), np.float32)
    for c in range(NCORES):
        out[c * SL:(c + 1) * SL] = res[c]["TOUT"]
    return out


def _conv_device(x, edge_index, h, alpha, c0, cj):
    key = "plans"
    row = edge_index[0].astype(np.int64)
    col = edge_index[1].astype(np.int64)
    if key not in _CACHE:
        new_of_old = _relabel(row, col)
        rr, cc = new_of_old[row], new_of_old[col]
        planP = _common_plan(src=rr, dst=cc)   # gather row -> scatter col
        planB = _common_plan(src=cc, dst=rr)   # gather col -> scatter row
        ncP = _build_transfer_nc(planP)
        ncB = _build_transfer_nc(planB)
        _CACHE[key] = (new_of_old, planP, planB,
                       _make_runner(ncP, replicated_names=("Zin",)), _make_runner(ncB, replicated_names=("Zin",)))
    new_of_old, planP, planB, runP, runB = _CACHE[key]

    deg = np.bincount(row, minlength=N).astype(np.float64)
    cj_c = cj[..., 0] + 1j * cj[..., 1]

    # relabeled state arrays [NPAD] (pads zero)
    xs = np.zeros((NPAD, H), np.float32)
    xs[new_of_old[:N]] = x
    degs = np.zeros(NPAD, np.float64)
    degs[new_of_old[:N]] = deg

    def cplx(a):   # [NPAD,128] f32 view from complex [NPAD,64]
        out = np.empty((NPAD, 128), np.float32)
        out[:, :64] = a.real
        out[:, 64:] = a.imag
        return out

    def uncplx(t):
        return (t[:, :64] + 1j * t[:, 64:]).astype(np.complex64)

    for l in range(NCONV):
        hl, al, c0l = float(h[l]), float(alpha[l]), float(c0[l])
        l_dia = degs - al
        tmp_left = 1.0 / (hl * l_dia + 1j)
        jac = (tmp_left * hl).astype(np.complex64)
        boff = (-tmp_left * hl).astype(np.complex64)
        b_dia = (tmp_left * (hl * l_dia - 1j)).astype(np.complex64)
        y = xs.astype(np.complex64)
        out = c0l * xs
        for j in range(R):
            t = uncplx(_transfer_dev(runB, planB, cplx(y)))
            b_j = boff[:, None] * t + b_dia[:, None] * y
            yk = b_j
            for _ in range(KK):
                z = jac[:, None] * yk
                yk = uncplx(_transfer_dev(runP, planP, cplx(z))) + b_j
            y = yk
            out = out + 2.0 * np.real(cj_c[l, j] * y)
        xs = np.maximum(out, 0.0)

    xf = np.empty((N, H), np.float64)
    xf = xs[new_of_old[:N]]
    return xf


# --------------------------------------------------------------------------
# fallbacks + head
# --------------------------------------------------------------------------

def _conv_numpy(x, edge_index, h, alpha, c0, cj):
    row, col = edge_index[0].astype(np.int64), edge_index[1].astype(np.int64)
    deg = np.bincount(row, minlength=N).astype(np.float64)
    cj_c = cj[..., 0] + 1j * cj[..., 1]
    x = x.astype(np.float64)
    for l in range(NCONV):
        hl, al, c0l = float(h[l]), float(alpha[l]), float(c0[l])
        l_dia = deg - al
        tmp_left = 1.0 / (hl * l_dia + 1j)
        jac = tmp_left * hl
        boff = -tmp_left * hl
        b_dia = tmp_left * (hl * l_dia - 1j)
        y = x.astype(np.complex128)
        out = c0l * x
        for j in range(R):
            t = np.zeros_like(y)
            np.add.at(t, row, y[col])
            b_j = boff[:, None] * t + b_dia[:, None] * y
            yk = b_j
            for _ in range(KK):
                z = jac[:, None] * yk
                t2 = np.zeros_like(y)
                np.add.at(t2, col, z[row])
                yk = t2 + b_j
            y = yk
            out = out + 2.0 * np.real(cj_c[l, j] * y)
        x = np.maximum(out, 0.0)
    return x


def _pool_head(x, batch, topk_w, lin_w, lin_b):
    s = np.tanh((x @ topk_w) / np.linalg.norm(topk_w))
    xp = x * s[:, None]
    k = int(np.ceil(RATIO * NPG))
    sg = s.reshape(G_GRAPHS, NPG)
    idx = np.argsort(-sg, axis=1, kind="stable")[:, :k]
    mask = np.zeros((G_GRAPHS, NPG), x.dtype)
    np.put_along_axis(mask, idx, 1.0, axis=1)
    pooled = (xp.reshape(G_GRAPHS, NPG, H) * mask[..., None]).sum(axis=1) / k
    return (pooled @ lin_w + lin_b).astype(np.float32)


def kernel(**inputs):
    x = np.asarray(inputs["x"], np.float32)
    edge_index = np.asarray(inputs["edge_index"])
    batch = np.asarray(inputs["batch"])
    h = np.asarray(inputs["h"], np.float32)
    alpha = np.asarray(inputs["alpha"], np.float32)
    c0 = np.asarray(inputs["c0"], np.float32)
    cj = np.asarray(inputs["cj"], np.float32)
    topk_w = np.asarray(inputs["topk_w"], np.float32)
    lin_w = np.asarray(inputs["lin_w"], np.float32)
    lin_b = np.asarray(inputs["lin_b"], np.float32)

    try:
        xf = _conv_device(x, edge_index, h, alpha, c0, cj)
    except Exception:
        import traceback
        traceback.print_exc()
        xf = _conv_numpy(x, edge_index, h, alpha, c0, cj)
    return _pool_head(xf, batch, topk_w, lin_w, lin_b)



# revision 11
# speedup vs baseline: 1.4365x; 1.4365x over previous
"""CayleyNet GNN kernel for Trainium2 — 8 NeuronCores, single fused NEFF.

Sharding (graph-parallel per the hint): nodes are band-sorted by P-direction
(dst=col) degree and dealt round-robin to 8 cores, so every core owns an
equal slice of destination nodes with matched degree profiles and one shared
SPMD program works for all cores.

The whole conv stack (2 layers x 3 Cayley orders x (1 B-step + 4 Jacobi
propagates) = 30 sparse transfers) runs in ONE device program:
  per transfer:
    - DVE computes the per-node complex-scaled source array z on the
      owning core's slice (z = jac (.) yk, or z = y for B-steps), cast to
      bf16, DMA'd to DRAM in two chunks.
    - Two AllGather collectives replicate the chunks into Za/Zb (halo
      exchange == full exchange for this random graph). Chunking keeps
      both arrays < 32768 rows (int16 gather-index reach) and lets the
      second AllGather overlap the first chunk's gathers.
    - CSR-by-destination gather: per-dst slot grids (tiles of 128 dsts,
      per-segment-uniform depth, lo/hi split by source chunk); gpsimd
      dma_gather fetches 256-byte rows; pad slots point at a zero row.
    - DVE segment tensor_reduce sums each dst's slots, then applies the
      CayleyNet update in SBUF (b = y - jac(.)(t + (2i/h) y) using
      bdia = 1 - (2i/h) jac; yk = t + b; out += 2 Re(cj yk)).
The tiny pooling head ([50000,64] -> [10,10]) runs on host.
"""
import numpy as np
import ml_dtypes

N = 50000
E = 800000
H = 64
G_GRAPHS = 10
NPG = N // G_GRAPHS
R = 3
KK = 4
NCONV = 2
OUT_DIM = 10
RATIO = 0.9

NCORES = 8
NTILE = 128
SL = 6272                 # nodes per core slice (49 tiles)
TPC = SL // NTILE         # 49
NPAD = SL * NCORES        # 50176
TILES_A = 25              # chunk-a tiles per core
TILES_B = TPC - TILES_A   # 24
SLA = TILES_A * NTILE     # 3200
SLB = TILES_B * NTILE     # 3072
ZA_ROWS = NCORES * (1 + SLA)  # 25608  (< 32768: int16 reach)
ZB_ROWS = NCORES * (1 + SLB)  # 24584  (rank stride includes a zero row)
SEGROWS = 96              # stg rows per buffer
GSEG = 16                 # max tiles per segment
CHUNK = 2048              # slots per dma_gather call

bf16 = ml_dtypes.bfloat16
_CACHE = {}


# --------------------------------------------------------------------------
# host graph preprocessing
# --------------------------------------------------------------------------

def _relabel(col):
    """Band-sort nodes by P-direction (dst=col) degree, deal round-robin to
    cores. Returns new_of_old [NPAD] -> relabeled id in [0, NPAD)."""
    degc = np.bincount(col, minlength=NPAD)[:NPAD]
    order = np.argsort(-degc, kind="stable")
    new_of_old = np.empty(NPAD, np.int64)
    b = np.arange(NPAD)
    new_of_old[order] = (b % NCORES) * SL + b // NCORES
    return new_of_old


def _build_plan(src, dst):
    """CSR-by-destination plan for one direction over relabeled endpoints.

    Slot layout: [all lo segments | all hi segments]; within a segment of g
    tiles at uniform depth D, tile k's slots are rows [k*D, (k+1)*D) x 128
    lanes. lo slots gather from Za (src loc < SLA), hi from Zb."""
    core = dst // SL
    loc = dst % SL
    tile = loc // NTILE
    lane = loc % NTILE
    sloc = src % SL
    score = src // SL
    is_lo = sloc < SLA
    zval = np.where(is_lo, score * (1 + SLA) + 1 + sloc,
                    score * (1 + SLB) + 1 + (sloc - SLA))
    assert zval.max() < 32768

    dall = np.zeros((NCORES, SL), np.int64)
    dlo = np.zeros((NCORES, SL), np.int64)
    np.add.at(dall, (core, loc), 1)
    np.add.at(dlo, (core, loc), is_lo.astype(np.int64))
    dhi = dall - dlo
    DLo_t = np.maximum(1, dlo.reshape(NCORES, TPC, NTILE).max(axis=(0, 2)))
    DHi_t = np.maximum(1, dhi.reshape(NCORES, TPC, NTILE).max(axis=(0, 2)))

    def mk_segs(Dt, base):
        segs = []
        t = 0
        while t < TPC:
            g, D = 1, int(Dt[t])
            assert D <= SEGROWS
            while (t + g < TPC and g < GSEG
                   and max(D, int(Dt[t + g])) * (g + 1) <= SEGROWS):
                D = max(D, int(Dt[t + g]))
                g += 1
            segs.append({"t0": t, "g": g, "D": D, "base": base})
            base += g * D * NTILE
            t += g
        return segs, base

    lo_segs, tot = mk_segs(DLo_t, 0)
    hi_segs, tot = mk_segs(DHi_t, tot)
    base_t = {"lo": np.zeros(TPC, np.int64), "hi": np.zeros(TPC, np.int64)}
    for which, segs in (("lo", lo_segs), ("hi", hi_segs)):
        for s in segs:
            for k in range(s["g"]):
                base_t[which][s["t0"] + k] = s["base"] + k * s["D"] * NTILE
    for segs, pas in ((lo_segs, "lo"), (hi_segs, "hi")):
        for s in segs:
            s["pass"] = pas
            end = s["base"] + s["g"] * s["D"] * NTILE
            calls = []
            p = s["base"]
            while p < end:
                n = min(CHUNK, end - p)
                calls.append((p, n))
                p += n
            s["calls"] = calls
            s["cols"] = (end - s["base"]) // 16

    # per-edge slot positions
    es = np.lexsort((np.where(is_lo, 0, 1), dst))
    s_core, s_loc = core[es], loc[es]
    s_tile, s_lane = tile[es], lane[es]
    s_lo, s_val = is_lo[es], zval[es]
    flat = s_core * SL + s_loc
    starts = np.zeros(NCORES * SL + 1, np.int64)
    np.cumsum(dall.reshape(-1), out=starts[1:])
    rank = np.arange(len(es)) - starts[flat]
    dlo_e = dlo.reshape(-1)[flat]
    pos = np.where(
        s_lo,
        base_t["lo"][s_tile] + rank * NTILE + s_lane,
        base_t["hi"][s_tile] + (rank - dlo_e) * NTILE + s_lane)

    idxs = []
    for c in range(NCORES):
        arr = np.zeros(tot, np.int16)
        m = s_core == c
        arr[pos[m]] = s_val[m].astype(np.int16)
        idxs.append(np.tile(arr.reshape(-1, 16).T, (8, 1)))

    return {"segs": lo_segs + hi_segs, "n_lo": len(lo_segs),
            "total": tot, "idx": np.stack(idxs)}


# --------------------------------------------------------------------------
# the fused device program
# --------------------------------------------------------------------------

def _transfer_list():
    lst = []
    for l in range(NCONV):
        for j in range(R):
            lst.append(("B", l, j, 0))
            for k in range(1, KK + 1):
                lst.append(("P", l, j, k))
    return lst


def _build_nc(planP, planB):
    import concourse.bacc as bacc
    import concourse.mybir as mybir
    dt = mybir.dt
    Alu = mybir.AluOpType
    nc = bacc.Bacc("TRN2", debug=False)

    X = nc.dram_tensor("X", [SL, 128], dt.float32, kind="ExternalInput")
    COEF = nc.dram_tensor("COEF", [128, 212], dt.float32, kind="ExternalInput")
    IXP = nc.dram_tensor("IXP", [128, planP["total"] // 16], dt.int16,
                         kind="ExternalInput")
    IXB = nc.dram_tensor("IXB", [128, planB["total"] // 16], dt.int16,
                         kind="ExternalInput")
    OUT = nc.dram_tensor("OUT", [SL, 64], dt.float32, kind="ExternalOutput")

    za_sl = [nc.dram_tensor(f"za_sl{p}", [1 + SLA, 128], dt.bfloat16,
                            kind="Internal") for p in range(2)]
    zb_sl = [nc.dram_tensor(f"zb_sl{p}", [1 + SLB, 128], dt.bfloat16,
                            kind="Internal") for p in range(2)]
    ZA = [nc.dram_tensor(f"ZA{p}", [ZA_ROWS, 128], dt.bfloat16, kind="Internal",
                         addr_space="Shared") for p in range(2)]
    ZB = [nc.dram_tensor(f"ZB{p}", [ZB_ROWS, 128], dt.bfloat16, kind="Internal",
                         addr_space="Shared") for p in range(2)]
    RG = [[0, 1, 2, 3, 4, 5, 6, 7]]

    transfers = _transfer_list()
    # global schedule: per transfer, per seg: global seg index + gather-call
    # cumulative counts + idx-load ordinal
    sched = []
    sb = 0
    gc = 0
    gafter_of_sb = []
    for i, tr in enumerate(transfers):
        plan = planB if tr[0] == "B" else planP
        segs = []
        for s in plan["segs"]:
            gc += len(s["calls"])
            segs.append({**s, "sb": sb, "gafter": gc})
            gafter_of_sb.append(gc)
            sb += 1
        # boundary seg: first hi seg covering tile TILES_A-1 (chunk-a tiles
        # all updated once its updates complete)
        bseg = None
        for q in range(plan["n_lo"], len(segs)):
            if segs[q]["t0"] + segs[q]["g"] >= TILES_A:
                bseg = q
                break
        assert bseg is not None and bseg < len(segs) - 1
        sched.append({"tr": tr, "plan": plan, "segs": segs,
                      "n_lo": plan["n_lo"], "bseg": bseg})
    n_segs_total = sb

    from contextlib import ExitStack
    with ExitStack() as _stk:
        e = _stk.enter_context
        block = e(nc.Block())
        y = e(nc.sbuf_tensor("y", [128, TPC, 128], dt.float32))
        b = e(nc.sbuf_tensor("b", [128, TPC, 128], dt.float32))
        zbf = e(nc.sbuf_tensor("zbf", [128, TPC, 128], dt.bfloat16))
        oacc = e(nc.sbuf_tensor("oacc", [128, TPC, 64], dt.float32))
        red_lo = e(nc.sbuf_tensor("red_lo", [128, TPC, 128], dt.float32))
        red_hi = e(nc.sbuf_tensor("red_hi", [128, GSEG, 128], dt.float32))
        t2 = e(nc.sbuf_tensor("t2", [128, GSEG, 64], dt.float32))
        t3 = e(nc.sbuf_tensor("t3", [128, GSEG, 64], dt.float32))
        stg = e(nc.sbuf_tensor("stg", [128, 2, SEGROWS, 128], dt.bfloat16))
        ixb = e(nc.sbuf_tensor("ixb", [128, 2, SEGROWS * 8], dt.int16))
        co = e(nc.sbuf_tensor("co", [128, 212], dt.float32))
        ztile = e(nc.sbuf_tensor("ztile", [128, 128], dt.bfloat16))
        s_ld = e(nc.semaphore("s_ld"))
        s_misc = e(nc.semaphore("s_misc"))
        s_zr = e(nc.semaphore("s_zr"))
        s_z = e(nc.semaphore("s_z"))
        s_sl = e(nc.semaphore("s_sl"))
        s_cc = e(nc.semaphore("s_cc"))
        s_ix = e(nc.semaphore("s_ix"))
        s_g = e(nc.semaphore("s_g"))
        s_r = e(nc.semaphore("s_r"))
        s_fin = e(nc.semaphore("s_fin"))
        s_out = e(nc.semaphore("s_out"))
        def jr_ap(l, t0, g):
            return co[:, l * TPC + t0: l * TPC + t0 + g][:, :, None] \
                .broadcast_to([128, g, 64])

        def ji_ap(l, t0, g):
            off = 2 * TPC
            return co[:, off + l * TPC + t0: off + l * TPC + t0 + g][:, :, None] \
                .broadcast_to([128, g, 64])

        def scal_ap(l, idx):
            off = 4 * TPC
            return co[:, off + l * 8 + idx: off + l * 8 + idx + 1]

        @block.sync
        def _(sp):
            sp.dma_start(y[:], X.rearrange("(a p) c -> p a c", p=128)
                         ).then_inc(s_ld, 16)
            sp.dma_start(co[:], COEF[:]).then_inc(s_ld, 16)
            sp.wait_ge(s_misc, 1)
            for arr in (za_sl[0], za_sl[1], zb_sl[0], zb_sl[1]):
                sp.dma_start(arr[0:1, :], ztile[0:1, 0:128]).then_inc(s_zr, 16)
            for i, ent in enumerate(sched):
                IX = IXB if ent["tr"][0] == "B" else IXP
                sp.wait_ge(s_z, 2 * i + 1)
                sp.dma_start(
                    za_sl[i % 2][1:1 + SLA, :].rearrange("(a p) c -> p a c",
                                                         p=128),
                    zbf[:, 0:TILES_A, :]).then_inc(s_sl, 16)
                sp.wait_ge(s_z, 2 * i + 2)
                sp.dma_start(
                    zb_sl[i % 2][1:1 + SLB, :].rearrange("(a p) c -> p a c",
                                                         p=128),
                    zbf[:, TILES_A:TPC, :]).then_inc(s_sl, 16)
                for s in ent["segs"]:
                    if s["sb"] >= 2:
                        sp.wait_ge(s_g, 16 * gafter_of_sb[s["sb"] - 2])
                    c0 = s["base"] // 16
                    sp.dma_start(ixb[:, s["sb"] % 2, 0:s["cols"]],
                                 IX[:, c0:c0 + s["cols"]]).then_inc(s_ix, 16)
            sp.wait_ge(s_fin, 1)
            sp.dma_start(OUT.rearrange("(a p) c -> p a c", p=128),
                         y[:, :, 0:64]).then_inc(s_out, 16)
            sp.wait_ge(s_out, 16)

        @block.gpsimd
        def _(gp):
            gp.wait_ge(s_zr, 64)
            for i, ent in enumerate(sched):
                SRC_A, SRC_B = ZA[i % 2], ZB[i % 2]
                gp.wait_ge(s_sl, 32 * i + 16)
                gp.collective_compute(
                    "AllGather", mybir.AluOpType.bypass, replica_groups=RG,
                    ins=[za_sl[i % 2][:]],
                    outs=[SRC_A[:]]).then_inc(s_cc, 1)
                gp.wait_ge(s_sl, 32 * i + 32)
                gp.collective_compute(
                    "AllGather", mybir.AluOpType.bypass, replica_groups=RG,
                    ins=[zb_sl[i % 2][:]],
                    outs=[SRC_B[:]]).then_inc(s_cc, 1)
                for q, s in enumerate(ent["segs"]):
                    if q == 0:
                        gp.wait_ge(s_cc, 2 * i + 1)
                    if q == ent["n_lo"]:
                        gp.wait_ge(s_cc, 2 * i + 2)
                    src = SRC_A if s["pass"] == "lo" else SRC_B
                    gp.wait_ge(s_ix, 16 * (s["sb"] + 1))
                    if s["sb"] >= 2:
                        gp.wait_ge(s_r, s["sb"] - 1)
                    for (ca, cn) in s["calls"]:
                        o = ca - s["base"]
                        iv = ixb[:, s["sb"] % 2, o // 16:(o + cn) // 16]
                        sv = stg[:, s["sb"] % 2, o // 128:(o + cn) // 128, :]
                        gp.dma_gather(sv, src[:], iv, cn, cn, 128
                                      ).then_inc(s_g, 16)

        @block.vector
        def _(ve):
            ve.memset(ztile[:], 0.0)
            ve.nop().then_inc(s_misc, 1)
            ve.wait_ge(s_ld, 32)
            ve.tensor_copy(zbf[:, 0:TILES_A, :], y[:, 0:TILES_A, :]
                           ).then_inc(s_z, 1)
            ve.tensor_copy(zbf[:, TILES_A:TPC, :], y[:, TILES_A:TPC, :]
                           ).then_inc(s_z, 1)
            ve.tensor_scalar(oacc[:], y[:, :, 0:64], scal_ap(0, 1), None,
                             Alu.mult)

            n_tr = len(sched)
            for i, ent in enumerate(sched):
                kind, l, j, k = ent["tr"]
                last_i = i == n_tr - 1
                for q, s in enumerate(ent["segs"]):
                    t0, g, D = s["t0"], s["g"], s["D"]
                    rows = g * D
                    buf = s["sb"] % 2
                    ve.wait_ge(s_g, 16 * s["gafter"])
                    inap = stg[:, buf, 0:rows, :].rearrange(
                        "p (g r) c -> p g r c", g=g).transpose([0, 1, 3, 2])
                    if s["pass"] == "lo":
                        ve.tensor_reduce(red_lo[:, t0:t0 + g, :], inap,
                                         mybir.AxisListType.X, Alu.add
                                         ).then_inc(s_r, 1)
                        continue
                    ve.tensor_reduce(red_hi[:, 0:g, :], inap,
                                     mybir.AxisListType.X, Alu.add
                                     ).then_inc(s_r, 1)
                    # t = lo + hi
                    ve.tensor_add(red_hi[:, 0:g, :], red_hi[:, 0:g, :],
                                  red_lo[:, t0:t0 + g, :])
                    RHr = red_hi[:, 0:g, 0:64]
                    RHi = red_hi[:, 0:g, 64:128]
                    yr = y[:, t0:t0 + g, 0:64]
                    yi = y[:, t0:t0 + g, 64:128]
                    T2, T3 = t2[:, 0:g, :], t3[:, 0:g, :]
                    jr, ji = jr_ap(l, t0, g), ji_ap(l, t0, g)
                    zr = zbf[:, t0:t0 + g, 0:64]
                    zi = zbf[:, t0:t0 + g, 64:128]
                    wrote_z = False
                    if kind == "B":
                        br = b[:, t0:t0 + g, 0:64]
                        bi = b[:, t0:t0 + g, 64:128]
                        h2 = scal_ap(l, 0)
                        # w = t + (2/h) i y  (in place in red_hi)
                        ve.tensor_scalar(T2, yi, h2, None, Alu.mult)
                        ve.tensor_sub(RHr, RHr, T2)
                        ve.tensor_scalar(T2, yr, h2, None, Alu.mult)
                        ve.tensor_add(RHi, RHi, T2)
                        # b = y - jac (.) w
                        ve.tensor_mul(T2, jr, RHr)
                        ve.tensor_mul(T3, ji, RHi)
                        ve.tensor_sub(T2, T2, T3)
                        ve.tensor_sub(br, yr, T2)
                        ve.tensor_mul(T2, jr, RHi)
                        ve.tensor_mul(T3, ji, RHr)
                        ve.tensor_add(T2, T2, T3)
                        ve.tensor_sub(bi, yi, T2)
                        # yk0 = b
                        ve.tensor_copy(y[:, t0:t0 + g, :], b[:, t0:t0 + g, :])
                        # z = jac (.) yk0
                        ve.tensor_mul(T2, jr, br)
                        ve.tensor_mul(T3, ji, bi)
                        ve.tensor_sub(zr, T2, T3)
                        ve.tensor_mul(T2, jr, bi)
                        ve.tensor_mul(T3, ji, br)
                        last = ve.tensor_add(zi, T2, T3)
                        wrote_z = True
                    else:
                        # yk = t + b
                        ve.tensor_add(y[:, t0:t0 + g, :], red_hi[:, 0:g, :],
                                      b[:, t0:t0 + g, :])
                        if k < KK:
                            ve.tensor_mul(T2, jr, yr)
                            ve.tensor_mul(T3, ji, yi)
                            ve.tensor_sub(zr, T2, T3)
                            ve.tensor_mul(T2, jr, yi)
                            ve.tensor_mul(T3, ji, yr)
                            last = ve.tensor_add(zi, T2, T3)
                            wrote_z = True
                        else:
                            ve.tensor_scalar(T2, yr, scal_ap(l, 2 + 2 * j),
                                             None, Alu.mult)
                            ve.tensor_scalar(T3, yi, scal_ap(l, 3 + 2 * j),
                                             None, Alu.mult)
                            ve.tensor_sub(T2, T2, T3)
                            ve.tensor_add(oacc[:, t0:t0 + g, :],
                                          oacc[:, t0:t0 + g, :], T2)
                            if j < R - 1:
                                last = ve.tensor_copy(zbf[:, t0:t0 + g, :],
                                                      y[:, t0:t0 + g, :])
                                wrote_z = True
                    if wrote_z and not last_i:
                        if q == ent["bseg"]:
                            last.then_inc(s_z, 1)
                        if q == len(ent["segs"]) - 1:
                            last.then_inc(s_z, 1)
                # layer-end (after j==R-1, k==KK transfer's segs)
                if kind == "P" and k == KK and j == R - 1:
                    if l == 0:
                        ve.tensor_scalar_max(y[:, :, 0:64], oacc[:], 0.0)
                        ve.tensor_scalar_mul(y[:, :, 64:128],
                                             y[:, :, 64:128], 0.0)
                        ve.tensor_scalar(oacc[:], y[:, :, 0:64],
                                         scal_ap(1, 1), None, Alu.mult)
                        ve.tensor_copy(zbf[:, 0:TILES_A, :],
                                       y[:, 0:TILES_A, :]).then_inc(s_z, 1)
                        ve.tensor_copy(zbf[:, TILES_A:TPC, :],
                                       y[:, TILES_A:TPC, :]).then_inc(s_z, 1)
                    else:
                        ve.tensor_scalar_max(y[:, :, 0:64], oacc[:], 0.0
                                             ).then_inc(s_fin, 1)

    nc.compile()
    return nc


# --------------------------------------------------------------------------
# SPMD runner (bass2jax via axon PJRT)
# --------------------------------------------------------------------------

def _make_runner(nc, n_cores=NCORES, replicated_names=()):
    import jax
    from jax.sharding import Mesh, PartitionSpec, NamedSharding
    from jax.experimental.shard_map import shard_map
    from concourse import mybir
    from concourse.bass2jax import (
        _bass_exec_p, install_neuronx_cc_hook, partition_id_tensor)

    install_neuronx_cc_hook()
    pname = nc.partition_id_tensor.name if nc.partition_id_tensor else None
    in_names, out_names, out_avals, zero_outs = [], [], [], []
    for alloc in nc.m.functions[0].allocations:
        if not isinstance(alloc, mybir.MemoryLocationSet):
            continue
        name = alloc.memorylocations[0].name
        if alloc.kind == "ExternalInput":
            if name != pname:
                in_names.append(name)
        elif alloc.kind == "ExternalOutput":
            shape = tuple(alloc.tensor_shape)
            dtype = mybir.dt.np(alloc.dtype)
            out_names.append(name)
            out_avals.append(jax.core.ShapedArray(shape, dtype))
            zero_outs.append(np.zeros(shape, dtype))
    n_outs = len(out_avals)
    all_in = list(in_names) + list(out_names) + ([pname] if pname else [])

    def _body(*args):
        operands = list(args)
        if pname is not None:
            operands.append(partition_id_tensor())
        outs = _bass_exec_p.bind(
            *operands, out_avals=tuple(out_avals), in_names=tuple(all_in),
            out_names=tuple(out_names), lowering_input_output_aliases=(),
            sim_require_finite=True, sim_require_nnan=True, nc=nc)
        return tuple(outs)

    try:
        devices = jax.devices("axon")[:n_cores]
    except Exception:
        devices = jax.devices()[:n_cores]
    mesh = Mesh(np.asarray(devices), ("core",))
    repl = set(replicated_names)
    in_specs = tuple(
        (PartitionSpec() if n in repl else PartitionSpec("core"))
        for n in in_names
    ) + (PartitionSpec("core"),) * n_outs
    sharded = jax.jit(
        shard_map(_body, mesh=mesh,
                  in_specs=in_specs,
                  out_specs=(PartitionSpec("core"),) * n_outs,
                  check_rep=False),
        keep_unused=True)

    sh = NamedSharding(mesh, PartitionSpec("core"))
    sh_rep = NamedSharding(mesh, PartitionSpec())
    dev_cache = {}

    def run(per_core_inputs, cache_names=()):
        concat_in = []
        for name in in_names:
            if name in dev_cache:
                concat_in.append(dev_cache[name])
                continue
            if name in repl:
                a = np.ascontiguousarray(np.asarray(per_core_inputs[0][name]))
                a = jax.device_put(a, sh_rep)
            else:
                a = np.ascontiguousarray(np.concatenate(
                    [np.asarray(per_core_inputs[c][name])
                     for c in range(n_cores)], axis=0))
                a = jax.device_put(a, sh)
            if name in cache_names:
                dev_cache[name] = a
            concat_in.append(a)
        if "_zeros" not in dev_cache:
            dev_cache["_zeros"] = [
                jax.device_put(
                    np.zeros((n_cores * z.shape[0], *z.shape[1:]), z.dtype), sh)
                for z in zero_outs
            ]
        outs = sharded(*concat_in, *dev_cache["_zeros"])
        outs = [np.asarray(a) for a in outs]
        return [
            {name: outs[i].reshape(n_cores, *out_avals[i].shape)[c]
             for i, name in enumerate(out_names)}
            for c in range(n_cores)
        ]
    return run


# --------------------------------------------------------------------------
# host orchestration
# --------------------------------------------------------------------------

def _prep(edge_index, h, alpha, c0, cj):
    row = edge_index[0].astype(np.int64)
    col = edge_index[1].astype(np.int64)
    if "plans" not in _CACHE:
        new_of_old = _relabel(col)
        rr, cc = new_of_old[row], new_of_old[col]
        planP = _build_plan(src=rr, dst=cc)
        planB = _build_plan(src=cc, dst=rr)
        nc = _build_nc(planP, planB)
        _CACHE["plans"] = (new_of_old, planP, planB, nc, _make_runner(nc))
    return _CACHE["plans"], row, col


def _coef_inputs(row, new_of_old, h, alpha, c0, cj):
    deg = np.bincount(row, minlength=N).astype(np.float64)
    degs = np.zeros(NPAD, np.float64)
    degs[new_of_old[:N]] = deg
    coefs = []
    for c in range(NCORES):
        co = np.zeros((128, 212), np.float32)
        coefs.append(co)
    for l in range(NCONV):
        hl, al = float(h[l]), float(alpha[l])
        u = hl * (degs - al)
        tmp_left = 1.0 / (u + 1j)
        jac = (tmp_left * hl).astype(np.complex64)
        for c in range(NCORES):
            sl = jac[c * SL:(c + 1) * SL].reshape(TPC, 128)
            coefs[c][:, l * TPC:(l + 1) * TPC] = sl.real.T
            coefs[c][:, 2 * TPC + l * TPC:2 * TPC + (l + 1) * TPC] = sl.imag.T
            off = 4 * TPC + l * 8
            coefs[c][:, off + 0] = 2.0 / hl
            coefs[c][:, off + 1] = float(c0[l])
            for j in range(R):
                coefs[c][:, off + 2 + 2 * j] = 2.0 * float(cj[l, j, 0])
                coefs[c][:, off + 3 + 2 * j] = 2.0 * float(cj[l, j, 1])
    return coefs


def _conv_device(x, edge_index, h, alpha, c0, cj):
    (new_of_old, planP, planB, nc, runner), row, col = _prep(
        edge_index, h, alpha, c0, cj)
    coefs = _coef_inputs(row, new_of_old, h, alpha, c0, cj)

    xs = np.zeros((NPAD, 128), np.float32)
    xs[new_of_old[:N], 0:64] = x
    maps = []
    for c in range(NCORES):
        maps.append({
            "X": xs[c * SL:(c + 1) * SL],
            "COEF": coefs[c],
            "IXP": planP["idx"][c],
            "IXB": planB["idx"][c],
        })
    import time as _time
    t0 = _time.perf_counter()
    res = runner(maps, cache_names=("IXP", "IXB"))
    _CACHE.setdefault("dev_times", []).append(_time.perf_counter() - t0)
    xf = np.empty((NPAD, 64), np.float32)
    for c in range(NCORES):
        xf[c * SL:(c + 1) * SL] = res[c]["OUT"]
    return xf[new_of_old[:N]]


# --------------------------------------------------------------------------
# numpy fallback + pooling head
# --------------------------------------------------------------------------

def _conv_numpy(x, edge_index, h, alpha, c0, cj):
    row, col = edge_index[0].astype(np.int64), edge_index[1].astype(np.int64)
    deg = np.bincount(row, minlength=N).astype(np.float64)
    cj_c = cj[..., 0] + 1j * cj[..., 1]
    x = x.astype(np.float64)
    for l in range(NCONV):
        hl, al, c0l = float(h[l]), float(alpha[l]), float(c0[l])
        l_dia = deg - al
        tmp_left = 1.0 / (hl * l_dia + 1j)
        jac = tmp_left * hl
        boff = -tmp_left * hl
        b_dia = tmp_left * (hl * l_dia - 1j)
        y = x.astype(np.complex128)
        out = c0l * x
        for j in range(R):
            t = np.zeros_like(y)
            np.add.at(t, row, y[col])
            b_j = boff[:, None] * t + b_dia[:, None] * y
            yk = b_j
            for _ in range(KK):
                z = jac[:, None] * yk
                t2 = np.zeros_like(y)
                np.add.at(t2, col, z[row])
                yk = t2 + b_j
            y = yk
            out = out + 2.0 * np.real(cj_c[l, j] * y)
        x = np.maximum(out, 0.0)
    return x


def _pool_head(x, batch, topk_w, lin_w, lin_b):
    s = np.tanh((x @ topk_w) / np.linalg.norm(topk_w))
    xp = x * s[:, None]
    k = int(np.ceil(RATIO * NPG))
    sg = s.reshape(G_GRAPHS, NPG)
    idx = np.argsort(-sg, axis=1, kind="stable")[:, :k]
    mask = np.zeros((G_GRAPHS, NPG), x.dtype)
    np.put_along_axis(mask, idx, 1.0, axis=1)
    pooled = (xp.reshape(G_GRAPHS, NPG, H) * mask[..., None]).sum(axis=1) / k
    return (pooled @ lin_w + lin_b).astype(np.float32)


def kernel(**inputs):
    x = np.asarray(inputs["x"], np.float32)
    edge_index = np.asarray(inputs["edge_index"])
    batch = np.asarray(inputs["batch"])
    h = np.asarray(inputs["h"], np.float32)
    alpha = np.asarray(inputs["alpha"], np.float32)
    c0 = np.asarray(inputs["c0"], np.float32)
    cj = np.asarray(inputs["cj"], np.float32)
    topk_w = np.asarray(inputs["topk_w"], np.float32)
    lin_w = np.asarray(inputs["lin_w"], np.float32)
    lin_b = np.asarray(inputs["lin_b"], np.float32)

    try:
        xf = _conv_device(x, edge_index, h, alpha, c0, cj)
    except Exception:
        import traceback
        traceback.print_exc()
        xf = _conv_numpy(x, edge_index, h, alpha, c0, cj)
    return _pool_head(xf.astype(np.float32), batch, topk_w, lin_w, lin_b)
